# revision 35
# baseline (speedup 1.0000x reference)
"""Trainium2 Bass kernel for nn_NeuralNetworkDPD (dense_mlp).

Hardware reality (measured): a cross-engine dependency hop costs ~2.3us
(semaphore wake-up), so throughput = hops_per_chunk * 2.3us / pipeline_depth.
The design therefore minimizes PSUM residency so MANY chunks can be in
flight at once (depth ~8-12), hiding the hop latency entirely:

  - Feature-major, 2-token-halves packed on 128 partitions (A-half rows
    {0,1} on partitions 0:64, B-half rows {2,3} on 64:128).
  - Centered carry: every dense stationary is W @ (I - J/64) so matmul
    outputs are pre-centered (LN mean subtraction is free). The final
    per-token mean is recovered with 4 rank-1 streams (feats, p1, p3, p5)
    accumulated into the w_out PSUM.
  - Every PSUM tile has exactly ONE consumer which immediately evacuates
    it to SBUF bf16:  vb = (dense_psum + bias)  [DVE tensor_scalar].
    Residual carry lives in SBUF:  c' = (dense_psum + bias) + c  [DVE stt].
  - Per layer: vb(V) -> vsq=vb*vb(V) -> va=ones' @ vsq(PE) ->
    rs=AbsRsqrt(va + eps/g^2)(S) -> w=vb*rs(V) -> p=Prelu(w+beta;alpha)(S)
    -> dense(PE).  gamma rides the stats stationary (1/(64 g_o^2)) so
    rs = gamma/sigma directly.
"""

import sys
from contextlib import ExitStack

sys.path.insert(0, "/opt/trn_rl_repo")

import numpy as np

import concourse.bacc as bacc
import concourse.bass as bass
import concourse.tile as tile
from concourse import mybir

F = 64          # feature width
NL = 6          # chained dense layers
EPS = 1e-3
CH = 512        # tokens-per-half per chunk (one PSUM bank)
G = 8           # chunks issued stage-blocked (pipeline depth)
BF = mybir.dt.bfloat16
F32 = mybir.dt.float32
ALU = mybir.AluOpType

PRELU_S_MOD = 1        # (chunk+1) % mod == 0 -> PReLU on ScalarE

# percol column layout
BR = 0          # read-bias for vb at k=0,1,3,5      (4 cols: idx by k)
BC = 6          # carry-add bias for k=1,3,5         (cols BC+k)
EG = 12         # eps/gamma_o^2 per layer            (6 cols)
BE = 18         # beta per layer                     (6 cols)
AL = 24         # alpha per layer                    (6 cols)
AB = 30         # alpha*beta per layer               (6 cols)
NPC = 36


def build_kernel(tc, outs, ins, tokens_per_row):
    nc = tc.nc
    TPR = tokens_per_row
    cpr = TPR // CH
    nchunks = 2 * cpr            # two row-pairs
    out = outs["out"]            # [4, 2, TPR] fp32 (planar re/im)
    xp = ins["xp"]               # [4, 2, TPR+3] bf16, host-padded (re/im)

    ctx = ExitStack()
    singles = ctx.enter_context(tc.tile_pool(name="singles", bufs=1))
    fpool = ctx.enter_context(tc.tile_pool(name="fpool", bufs=6))
    vbpool = ctx.enter_context(tc.tile_pool(name="vb", bufs=G + 2))
    vqpool = ctx.enter_context(tc.tile_pool(name="vq", bufs=G + 2))
    rspool = ctx.enter_context(tc.tile_pool(name="rs", bufs=G + 2))
    wpool = ctx.enter_context(tc.tile_pool(name="w", bufs=G + 2))
    qpool = ctx.enter_context(tc.tile_pool(name="q", bufs=G + 2))
    cpool = ctx.enter_context(tc.tile_pool(name="cp", bufs=2 * G + 2))
    ptpool = ctx.enter_context(tc.tile_pool(name="pt", bufs=G + 2))
    pkpool = ctx.enter_context(tc.tile_pool(name="pk", bufs=3 * G + 3))
    otpool = ctx.enter_context(tc.tile_pool(name="ot", bufs=4))
    bpool = ctx.enter_context(tc.tile_pool(name="bp", bufs=4, space="PSUM"))
    vapool = ctx.enter_context(tc.tile_pool(name="va", bufs=4, space="PSUM"))

    # ---- load weights/constants into SBUF ----
    wd = singles.tile([128, NL * 128], BF)
    ones6 = singles.tile([128, NL * 128], BF)
    win = singles.tile([16, 128], BF)
    wtail = singles.tile([128, 4], BF)
    fext = singles.tile([16, 4], BF)
    pext = singles.tile([128, 12], BF)
    percol = singles.tile([128, NPC], F32)
    nc.sync.dma_start(out=wd, in_=ins["wd"])
    nc.sync.dma_start(out=ones6, in_=ins["ones6"])
    nc.sync.dma_start(out=win, in_=ins["win"])
    nc.sync.dma_start(out=wtail, in_=ins["wtail"])
    nc.sync.dma_start(out=fext, in_=ins["fext"])
    nc.sync.dma_start(out=pext, in_=ins["pext"])
    nc.sync.dma_start(out=percol, in_=ins["percol"])

    col = lambda base, k: percol[:, base + k: base + k + 1]

    def chunk_rowt(f):
        rp, ci = f // cpr, f % cpr
        return rp, 2 + rp, ci * CH

    state = {}

    LB = 4                       # chunks sharing one feats/out DMA

    def stage_load(f):
        # one DMA loads feats for LB consecutive chunks (same row-pair)
        if f % LB != 0:
            fb = state[f - f % LB]["featsb"]
            state[f] = {"feats": fb[:, (f % LB) * CH:(f % LB + 1) * CH],
                        "featsb": fb, "pk": {}}
            return
        rowA, rowB, t0 = chunk_rowt(f)
        T = TPR + 3
        fb = fpool.tile([16, LB * CH], BF, tag="feats", name=f"feats{f}")
        # xp is [2(ri), 4(row), T]; partition p = ri*8 + half*4 + lag
        # merged dim (ri,half): stride 2T count 4; lag: stride 1 count 4
        src = bass.AP(tensor=xp.tensor,
                      offset=rowA * T + t0,
                      ap=[[2 * T, 4], [1, 4], [1, LB * CH]])
        nc.sync.dma_start(out=fb, in_=src)
        state[f] = {"feats": fb[:, 0:CH], "featsb": fb, "pk": {}}

    def stage_win(f):
        st = state[f]
        b = bpool.tile([128, CH], F32, tag="b", name=f"z0_{f}")
        nc.tensor.matmul(out=b, lhsT=win, rhs=st["feats"],
                         start=True, stop=True)
        st["b"] = b

    def stage_vb(f, k):
        """Evacuate the dense/win psum (single reader) or alias the carry."""
        st = state[f]
        if k in (2, 4):
            st["vb"] = st["carry"]          # bias already folded in
            return
        vb = vbpool.tile([128, CH], BF, tag="vb", name=f"vb{f}_{k}")
        nc.vector.tensor_scalar_add(vb, st["b"], col(BR, k))
        st["vb"] = vb
        if k == 0:
            st["carry"] = vb                # c0 = z0 + b_in

    def stage_vsq(f, k):
        st = state[f]
        vsq = vqpool.tile([128, CH], BF, tag="vsq", name=f"vsq{f}_{k}")
        nc.vector.tensor_tensor(out=vsq, in0=st["vb"], in1=st["vb"],
                                op=ALU.mult)
        st["vsq"] = vsq

    def stage_va(f, k):
        st = state[f]
        va = vapool.tile([128, CH], F32, tag="va", name=f"va{f}_{k}")
        nc.tensor.matmul(out=va, lhsT=ones6[:, k * 128:(k + 1) * 128],
                         rhs=st["vsq"], start=True, stop=True)
        st["va"] = va

    def stage_rs(f, k):
        st = state[f]
        rs = rspool.tile([128, CH], BF, tag="rs", name=f"rs{f}_{k}")
        nc.scalar.activation(
            out=rs, in_=st["va"],
            func=mybir.ActivationFunctionType.Abs_reciprocal_sqrt,
            bias=col(EG, k), scale=1.0)
        st["rs"] = rs

    def stage_w(f, k):
        st = state[f]
        w = wpool.tile([128, CH], BF, tag="w", name=f"w{f}_{k}")
        nc.vector.tensor_tensor(out=w, in0=st["vb"], in1=st["rs"],
                                op=ALU.mult)
        st["w"] = w

    def stage_prelu(f, k):
        st = state[f]
        w = st["w"]
        pool = pkpool if k % 2 == 1 else ptpool
        p = pool.tile([128, CH], BF, tag="p", name=f"p{f}_{k}")
        if (f + 1) % PRELU_S_MOD == 0:
            nc.scalar.activation(out=p, in_=w,
                                 func=mybir.ActivationFunctionType.Prelu,
                                 bias=col(BE, k), scale=1.0, alpha=col(AL, k))
        else:
            # p = max(w + beta, alpha*w + alpha*beta); valid for alpha <= 1
            q = qpool.tile([128, CH], BF, tag="q", name=f"q{f}_{k}")
            nc.vector.tensor_scalar(out=q, in0=w, scalar1=col(AL, k),
                                    scalar2=col(AB, k), op0=ALU.mult,
                                    op1=ALU.add)
            t = qpool.tile([128, CH], BF, tag="q", name=f"t{f}_{k}")
            nc.vector.tensor_scalar(out=t, in0=w, scalar1=col(BE, k),
                                    scalar2=None, op0=ALU.add)
            nc.vector.tensor_tensor(out=p, in0=t, in1=q, op=ALU.max)
        if k % 2 == 1:
            st["pk"][k] = p
        st["p"] = p

    def stage_dense(f, k):
        st = state[f]
        b = bpool.tile([128, CH], F32, tag="b", name=f"b{f}_{k}")
        nc.tensor.matmul(out=b, lhsT=wd[:, k * 128:(k + 1) * 128],
                         rhs=st["p"], start=True, stop=True)
        st["b"] = b

    def stage_carry(f, k):
        """After dense k in {1,3,5}: c' = (dense_psum + bc_k) + c."""
        st = state[f]
        c = cpool.tile([128, CH], BF, tag="c", name=f"c{f}_{k}")
        nc.vector.scalar_tensor_tensor(out=c, in0=st["b"], scalar=col(BC, k),
                                       in1=st["carry"], op0=ALU.add,
                                       op1=ALU.add)
        st["carry"] = c

    def tail_opmm(f):
        st = state[f]
        op = vapool.tile([4, CH], F32, tag="va", padded_shape=[128, CH],
                         name=f"op{f}")
        nc.tensor.matmul(out=op, lhsT=wtail, rhs=st["carry"],
                         start=True, stop=False, skip_group_check=True)
        nc.tensor.matmul(out=op, lhsT=fext, rhs=st["feats"],
                         start=False, stop=False, skip_group_check=True)
        for j, k in enumerate((1, 3, 5)):
            nc.tensor.matmul(out=op, lhsT=pext[:, 4 * j: 4 * j + 4],
                             rhs=st["pk"][k], start=False, stop=(k == 5),
                             skip_group_check=True)
        st["op"] = op

    otb = {}

    def tail_store(f):
        st = state[f]
        rowA, rowB, t0 = chunk_rowt(f)
        j = f % LB
        if j == 0:
            otb[0] = otpool.tile([4, LB * CH], F32, tag="ot", name=f"ot{f}")
        ot = otb[0]
        nc.scalar.copy(out=ot[:, j * CH:(j + 1) * CH], in_=st["op"])
        if j == LB - 1:
            # out is planar [4, 2, TPR]; partitions (half:2) x (re/im:2)
            dst = bass.AP(tensor=out.tensor,
                          offset=rowA * 2 * TPR + (t0 - (LB - 1) * CH),
                          ap=[[4 * TPR, 2], [TPR, 2], [1, LB * CH]])
            nc.sync.dma_start(out=dst, in_=ot)
        del state[f]

    def emit_layer(grp, k):
        for f in grp:
            stage_vb(f, k)
        for f in grp:
            stage_vsq(f, k)
        for f in grp:
            stage_va(f, k)
        for f in grp:
            stage_rs(f, k)
        for f in grp:
            stage_w(f, k)
        for f in grp:
            stage_prelu(f, k)
        for f in grp:
            stage_dense(f, k)
        if k in (1, 3, 5):
            for f in grp:
                stage_carry(f, k)

    # ---- main loop: groups of G chunks, tails overlapped with the next
    # group's first layer ----
    groups = [list(range(f0, min(f0 + G, nchunks)))
              for f0 in range(0, nchunks, G)]
    prev = None
    for grp in groups:
        for f in grp:
            stage_load(f)
        for f in grp:
            stage_win(f)
        emit_layer(grp, 0)
        if prev is not None:
            for f in prev:
                tail_opmm(f)
            for f in prev:
                tail_store(f)
        for k in range(1, NL):
            emit_layer(grp, k)
        prev = grp
    for f in prev:
        tail_opmm(f)
    for f in prev:
        tail_store(f)
    ctx.close()


def _host_pack(inputs):
    """Build the shared (replicated) packed-weight arrays."""
    w_in = np.asarray(inputs["w_in"], np.float32)
    dense_w = np.asarray(inputs["dense_w"], np.float32)
    w_out = np.asarray(inputs["w_out"], np.float32)
    ln_gamma = np.asarray(inputs["ln_gamma"], np.float32)
    ln_beta = np.asarray(inputs["ln_beta"], np.float32)
    alpha = np.asarray(inputs["alpha"], np.float32)
    b_in = np.asarray(inputs["b_in"], np.float32)
    dense_b = np.asarray(inputs["dense_b"], np.float32)

    C = np.eye(F, dtype=np.float32) - 1.0 / F   # centering projector

    # feats partition order: p = ri*8 + half*4 + lag (ri = re/im)
    win = np.zeros((16, 128), np.float32)
    winC = w_in @ C
    for ri in range(2):
        for half in range(2):
            for lag in range(4):
                p = ri * 8 + half * 4 + lag
                win[p, half * 64:(half + 1) * 64] = winC[ri * 4 + lag]

    wd = np.zeros((128, NL * 128), np.float32)
    ones6 = np.zeros((128, NL * 128), np.float32)
    for l in range(NL):
        wdC = dense_w[l] @ C
        wd[0:64, l * 128: l * 128 + 64] = wdC
        wd[64:128, l * 128 + 64: l * 128 + 128] = wdC
        g2 = ln_gamma[l] ** 2                     # [F]
        blk = np.repeat((1.0 / (F * g2))[None, :], F, axis=0)  # [F_in, F_out]
        ones6[0:64, l * 128: l * 128 + 64] = blk
        ones6[64:128, l * 128 + 64: l * 128 + 128] = blk

    wtail = np.zeros((128, 4), np.float32)
    wtail[0:64, 0:2] = w_out
    wtail[64:128, 2:4] = w_out

    s = w_out.sum(axis=0)                         # [2]
    wbar_in = w_in.mean(axis=1)                   # [8]
    wbs = np.outer(wbar_in, s)                    # [8, 2]
    fext = np.zeros((16, 4), np.float32)
    for ri in range(2):
        for half in range(2):
            for lag in range(4):
                p = ri * 8 + half * 4 + lag
                fext[p, half * 2: half * 2 + 2] = wbs[ri * 4 + lag]
    pext = np.zeros((128, 12), np.float32)
    for j, l in enumerate((1, 3, 5)):
        wbar = dense_w[l].mean(axis=1)            # [F]
        pext[0:64, 4 * j: 4 * j + 2] = np.outer(wbar, s)
        pext[64:128, 4 * j + 2: 4 * j + 4] = np.outer(wbar, s)

    # biases (centered-carry bookkeeping)
    bc_in = C @ b_in
    bc = [C @ dense_b[l] for l in range(NL)]
    vb_bias = {0: bc_in, 1: bc[0], 3: bc[2], 5: bc[4]}

    percol = np.zeros((128, NPC), np.float32)
    for k, v in vb_bias.items():
        percol[:, BR + k] = np.tile(v, 2)
    for k in (1, 3, 5):
        percol[:, BC + k] = np.tile(bc[k], 2)
    for k in range(NL):
        g = ln_gamma[k]
        percol[:, EG + k] = np.tile(EPS / (g * g), 2)
        percol[:, BE + k] = np.tile(ln_beta[k], 2)
        percol[:, AL + k] = np.tile(alpha[k], 2)
        percol[:, AB + k] = np.tile(alpha[k] * ln_beta[k], 2)

    # constant part of the final mean correction, folded into b_out
    m_const = b_in.mean() + dense_b[1].mean() + dense_b[3].mean() \
        + dense_b[5].mean()
    b_out_eff = np.asarray(inputs["b_out"], np.float32) + m_const * s

    bf_np = mybir.dt.np(BF)
    shared = dict(wd=wd, ones6=ones6, win=win, wtail=wtail, fext=fext,
                  pext=pext)
    shared = {k: np.ascontiguousarray(v.astype(bf_np))
              for k, v in shared.items()}
    shared["percol"] = percol
    return shared, b_out_eff


def _prep_x(xr, xi, dtype):
    """Pack [2, 4, TPR+3]: (re/im) x rows x left-padded-3 samples, bf16."""
    xr = np.pad(np.asarray(xr, np.float32), ((0, 0), (3, 0)))
    xi = np.pad(np.asarray(xi, np.float32), ((0, 0), (3, 0)))
    return np.ascontiguousarray(np.stack([xr, xi], axis=0).astype(dtype))


def build_program(tokens_per_row):
    """Build the full Bass/Tile program for one core's shard."""
    nc = bacc.Bacc("TRN2")
    ins = {}
    shapes = dict(wd=(128, NL * 128), ones6=(128, NL * 128), win=(16, 128),
                  wtail=(128, 4), fext=(16, 4), pext=(128, 12),
                  percol=(128, NPC))
    for name, shp in shapes.items():
        dt = F32 if name == "percol" else BF
        ins[name] = nc.dram_tensor(name, list(shp), dt,
                                   kind="ExternalInput").ap()
    ins["xp"] = nc.dram_tensor("xp", [2, 4, tokens_per_row + 3], BF,
                               kind="ExternalInput").ap()
    outs = {"out": nc.dram_tensor("out", [4, 2, tokens_per_row],
                                  F32, kind="ExternalOutput").ap()}
    with tile.TileContext(nc) as tc:
        build_kernel(tc, outs, ins, tokens_per_row)
    nc.compile()
    return nc


def _run(inputs, trace=False):
    from concourse.bass_utils import run_bass_kernel_spmd

    x_real = np.asarray(inputs["x_real"], np.float32)
    x_imag = np.asarray(inputs["x_imag"], np.float32)
    B, N = x_real.shape
    n_cores = 8
    rows_per_core = B // n_cores

    shared, b_out_eff = _host_pack(inputs)
    nc = build_program(N)
    bf_np = mybir.dt.np(BF)

    in_maps = []
    for c in range(n_cores):
        m = dict(shared)
        sl = slice(c * rows_per_core, (c + 1) * rows_per_core)
        m["xp"] = _prep_x(x_real[sl], x_imag[sl], bf_np)
        in_maps.append(m)

    res = run_bass_kernel_spmd(nc, in_maps, core_ids=list(range(n_cores)),
                               trace=trace)
    outs_np = [r["out"] for r in res.results]
    full = np.concatenate(outs_np, axis=0)          # [B, 2, N]
    re = full[:, 0, :] + b_out_eff[0] + x_real
    im = full[:, 1, :] + b_out_eff[1] + x_imag
    return (re + 1j * im).astype(np.complex64), res


def kernel(**inputs):
    return _run(inputs, trace=False)[0]
